# revision 7
# baseline (speedup 1.0000x reference)
"""AttentionCropLayer on 8 TRN2 NeuronCores, pure data-parallel.

Math: the whole layer is separable per image. With
  mx[x] = sigmoid(10(x-tx+tl)) - sigmoid(10(x-tx-tl))   (mask rows)
  my[y] likewise for ty, and the bilinear crop+resize being a sparse
  linear map per axis (two taps per output row), the output is
  out[c] = A @ img[c] @ B  where
  A[j,x]  = relu(1-|sA[j]-x|) * mx[x]   (tent = bilinear weights)
  B[y,jy] = relu(1-|sB[jy]-y|) * my[y]
  sA[j] = (tx-tl) + clamp((j+0.5)*2tl/224 - 0.5, 0, 2tl-1).
No data-dependent addressing: per-image matrices are built on-device with
elementwise ops from apn_out, then two bf16 matmul stages per channel.
"""

import sys

sys.path.insert(0, "/opt/trn_rl_repo")

import numpy as np

import concourse.bass as bass
import concourse.tile as tile
from concourse import bacc, mybir
from concourse.bass_utils import run_bass_kernel_spmd

F32 = mybir.dt.float32
BF16 = mybir.dt.bfloat16

NCORES = 8
B_FULL = 256
BL = B_FULL // NCORES  # 32 images per core
IMG = 224
H = 112  # chunk size (224 = 2*112)
MAGIC = 12582912.0  # 1.5 * 2^23: float32 round-to-int magic

_cache = {}


def _build_graph():
    nc = bacc.Bacc("TRN2", target_bir_lowering=False, debug=False)

    apn_d = nc.dram_tensor("apn", [BL, 3], F32, kind="ExternalInput")
    img_d = nc.dram_tensor("img", [BL, 3, IMG, IMG], F32, kind="ExternalInput")
    jh_d = nc.dram_tensor("jh_mat", [H, IMG], F32, kind="ExternalInput")
    xcol_d = nc.dram_tensor("xcol", [H, 2], F32, kind="ExternalInput")
    out_d = nc.dram_tensor("out", [BL, 3, IMG, IMG], F32, kind="ExternalOutput")

    with tile.TileContext(nc) as tc:
        with (
            tc.tile_pool(name="const", bufs=1) as constp,
            tc.tile_pool(name="param", bufs=1) as parp,
            tc.tile_pool(name="imgp", bufs=2) as imgp,
            tc.tile_pool(name="sbld", bufs=2) as sbld,
            tc.tile_pool(name="tent", bufs=3) as tentp,
            tc.tile_pool(name="mats", bufs=2) as matsp,
            tc.tile_pool(name="tbf", bufs=3) as tbfp,
            tc.tile_pool(name="outp", bufs=2) as outp,
            tc.tile_pool(name="ps_t", bufs=3, space=bass.MemorySpace.PSUM) as ps_t,
            tc.tile_pool(name="ps_o", bufs=3, space=bass.MemorySpace.PSUM) as ps_o,
        ):
            # ---- constants ----
            jh_mat = constp.tile([H, IMG], F32, tag="jh", name="jh")  # (j+0.5)/112, all parts
            nc.sync.dma_start(jh_mat[:], jh_d.ap())
            xcol = constp.tile([H, 2], F32, tag="xc", name="xc")  # xcol[p,k] = p + 112k
            nc.sync.dma_start(xcol[:], xcol_d.ap())

            # ---- per-image parameters, row layout [1, BL] ----
            apn_rows = []
            for pi in range(3):
                ar = parp.tile([1, BL], F32, tag=f"apn_r{pi}", name=f"apn_r{pi}")
                nc.sync.dma_start(ar[:], apn_d.ap().transpose([1, 0])[pi:pi + 1, :])
                apn_rows.append(ar)

            def row(tag):
                return parp.tile([1, BL], F32, tag=tag, name=tag)

            v = nc.vector

            def trunc_rows(dst, a_row, m0, m1, floor_only=False):
                # dst = trunc(op1(op0(a)))-style helper: dst = trunc(a*m0 + m1)
                u = row(dst + "_u")
                v.tensor_scalar(u[:], a_row, m0, m1, mybir.AluOpType.mult,
                                mybir.AluOpType.add)
                f = row(dst + "_f")
                v.tensor_scalar(f[:], u[:], 0.5, MAGIC, mybir.AluOpType.subtract,
                                mybir.AluOpType.add)
                v.tensor_scalar(f[:], f[:], MAGIC, None, mybir.AluOpType.subtract,
                                mybir.AluOpType.bypass)
                if not floor_only:
                    m = row(dst + "_m")
                    v.tensor_scalar(m[:], u[:], -1048576.0, 0.0,
                                    mybir.AluOpType.mult, mybir.AluOpType.max)
                    v.tensor_scalar(m[:], m[:], 1.0, None, mybir.AluOpType.min,
                                    mybir.AluOpType.bypass)
                    v.tensor_add(f[:], f[:], m[:])
                return f

            # tx = 112 + trunc(a0*56+0.5); ty likewise; tl = 38 + floor((a2+1)*9)
            tx = trunc_rows("tx", apn_rows[0][:], 56.0, 0.5)
            v.tensor_scalar(tx[:], tx[:], 112.0, None, mybir.AluOpType.add,
                            mybir.AluOpType.bypass)
            ty = trunc_rows("ty", apn_rows[1][:], 56.0, 0.5)
            v.tensor_scalar(ty[:], ty[:], 112.0, None, mybir.AluOpType.add,
                            mybir.AluOpType.bypass)
            # (a2+1)*9: op0=add 1, op1=mult 9 (order matters for f32 parity)
            tl_u = row("tl_u")
            v.tensor_scalar(tl_u[:], apn_rows[2][:], 1.0, 9.0, mybir.AluOpType.add,
                            mybir.AluOpType.mult)
            tl = row("tl")
            v.tensor_scalar(tl[:], tl_u[:], 0.5, MAGIC, mybir.AluOpType.subtract,
                            mybir.AluOpType.add)
            v.tensor_scalar(tl[:], tl[:], MAGIC, None, mybir.AluOpType.subtract,
                            mybir.AluOpType.bypass)
            v.tensor_scalar(tl[:], tl[:], 38.0, None, mybir.AluOpType.add,
                            mybir.AluOpType.bypass)

            stx = row("stx")
            v.tensor_sub(stx[:], tx[:], tl[:])
            enx = row("enx")
            v.tensor_add(enx[:], tx[:], tl[:])
            sty = row("sty")
            v.tensor_sub(sty[:], ty[:], tl[:])
            eny = row("eny")
            v.tensor_add(eny[:], ty[:], tl[:])
            t2m1 = row("t2m1")
            v.tensor_scalar(t2m1[:], tl[:], 2.0, 1.0, mybir.AluOpType.mult,
                            mybir.AluOpType.subtract)

            # ---- broadcast params to all 112 partitions ----
            def bcast(r, tag):
                t = parp.tile([H, BL], F32, tag=tag, name=tag)
                nc.gpsimd.partition_broadcast(t[:], r[:])
                return t

            TLb = bcast(tl, "TLb")
            T2b = bcast(t2m1, "T2b")
            STXb = bcast(stx, "STXb")
            ENXb = bcast(enx, "ENXb")
            STYb = bcast(sty, "STYb")
            ENYb = bcast(eny, "ENYb")

            # ---- mask columns mx/my [H, 2, BL]; plus negated copies ----
            def mask_cols(STb, ENb, tag):
                m = parp.tile([H, 2, BL], F32, tag=tag, name=tag)
                for k in range(2):
                    d1 = parp.tile([H, BL], F32, tag="mc_d1", name="mc_d1")
                    v.tensor_scalar(d1[:], STb[:], xcol[:, k:k + 1], -3.0,
                                    mybir.AluOpType.subtract, mybir.AluOpType.max)
                    v.tensor_scalar(d1[:], d1[:], 3.0, None, mybir.AluOpType.min,
                                    mybir.AluOpType.bypass)
                    g1 = parp.tile([H, BL], F32, tag="mc_g1", name="mc_g1")
                    nc.scalar.activation(g1[:], d1[:],
                                         mybir.ActivationFunctionType.Sigmoid,
                                         scale=-10.0)
                    d2 = parp.tile([H, BL], F32, tag="mc_d2", name="mc_d2")
                    v.tensor_scalar(d2[:], ENb[:], xcol[:, k:k + 1], -3.0,
                                    mybir.AluOpType.subtract, mybir.AluOpType.max)
                    v.tensor_scalar(d2[:], d2[:], 3.0, None, mybir.AluOpType.min,
                                    mybir.AluOpType.bypass)
                    g2 = parp.tile([H, BL], F32, tag="mc_g2", name="mc_g2")
                    nc.scalar.activation(g2[:], d2[:],
                                         mybir.ActivationFunctionType.Sigmoid,
                                         scale=-10.0)
                    v.tensor_sub(m[:, k, :], g1[:], g2[:])
                neg = parp.tile([H, 2, BL], F32, tag=tag + "_n", name=tag + "_n")
                v.tensor_scalar(neg[:], m[:], -1.0, None, mybir.AluOpType.mult,
                                mybir.AluOpType.bypass)
                return m, neg

            mx, nmx = mask_cols(STXb, ENXb, "mx")
            my, nmy = mask_cols(STYb, ENYb, "my")

            # ---- per-image pipeline ----
            drain_ctr = [0]

            def drain_copy(out_ap, in_ap):
                # alternate PSUM drains between DVE and ACT
                drain_ctr[0] += 1
                if drain_ctr[0] % 2:
                    nc.vector.tensor_copy(out_ap, in_ap)
                else:
                    nc.scalar.activation(out_ap, in_ap,
                                         mybir.ActivationFunctionType.Copy)

            for b in range(BL):
                # image load + bf16 convert (gpsimd)
                img_f = imgp.tile([H, 3, 2, IMG], F32, tag="img_f", name="img_f")
                nc.sync.dma_start(
                    img_f[:],
                    img_d.ap()[b].rearrange("c (k p) y -> p c k y", k=2),
                )
                img_bf = imgp.tile([H, 3, 2, IMG], BF16, tag="img_bf", name="img_bf")
                nc.gpsimd.tensor_copy(img_bf[:], img_f[:])

                # s rows (shared u), built on gpsimd: [H, IMG] broadcast form
                uu = sbld.tile([H, IMG], F32, tag="uu", name="uu")
                nc.gpsimd.tensor_scalar(uu[:], jh_mat[:], TLb[:, b:b + 1], None,
                                        mybir.AluOpType.mult,
                                        mybir.AluOpType.bypass)
                nc.gpsimd.tensor_scalar(uu[:], uu[:], 0.5, 0.0,
                                        mybir.AluOpType.subtract,
                                        mybir.AluOpType.max)
                sA = sbld.tile([H, IMG], F32, tag="sA", name="sA")
                nc.gpsimd.tensor_scalar(sA[:], uu[:], T2b[:, b:b + 1],
                                        STXb[:, b:b + 1], mybir.AluOpType.min,
                                        mybir.AluOpType.add)
                sB = sbld.tile([H, IMG], F32, tag="sB", name="sB")
                nc.gpsimd.tensor_scalar(sB[:], uu[:], T2b[:, b:b + 1],
                                        STYb[:, b:b + 1], mybir.AluOpType.min,
                                        mybir.AluOpType.add)

                # tents + mask fold -> A^T [H,2,IMG] bf16, B [H,2,IMG] bf16
                AT = matsp.tile([H, 2, IMG], BF16, tag="AT", name="AT")
                Bm = matsp.tile([H, 2, IMG], BF16, tag="Bm", name="Bm")
                for k in range(2):
                    # tent*m = min(relu(m*(1-d)), relu(m*(1+d))), d = s - x
                    dA = tentp.tile([H, IMG], F32, tag="dA", name="dA")
                    v.tensor_scalar(dA[:], sA[:], xcol[:, k:k + 1], None,
                                    mybir.AluOpType.subtract,
                                    mybir.AluOpType.bypass)
                    uA1 = tentp.tile([H, IMG], F32, tag="uA1", name="uA1")
                    nc.scalar.activation(uA1[:], dA[:],
                                         mybir.ActivationFunctionType.Relu,
                                         bias=mx[:, k, b:b + 1],
                                         scale=nmx[:, k, b:b + 1])
                    uA2 = tentp.tile([H, IMG], F32, tag="uA2", name="uA2")
                    nc.scalar.activation(uA2[:], dA[:],
                                         mybir.ActivationFunctionType.Relu,
                                         bias=mx[:, k, b:b + 1],
                                         scale=mx[:, k, b:b + 1])
                    v.tensor_tensor(AT[:, k, :], uA1[:], uA2[:],
                                    mybir.AluOpType.min)
                    dB = tentp.tile([H, IMG], F32, tag="dB", name="dB")
                    v.tensor_scalar(dB[:], sB[:], xcol[:, k:k + 1], None,
                                    mybir.AluOpType.subtract,
                                    mybir.AluOpType.bypass)
                    uB1 = tentp.tile([H, IMG], F32, tag="uB1", name="uB1")
                    nc.scalar.activation(uB1[:], dB[:],
                                         mybir.ActivationFunctionType.Relu,
                                         bias=my[:, k, b:b + 1],
                                         scale=nmy[:, k, b:b + 1])
                    uB2 = tentp.tile([H, IMG], F32, tag="uB2", name="uB2")
                    nc.scalar.activation(uB2[:], dB[:],
                                         mybir.ActivationFunctionType.Relu,
                                         bias=my[:, k, b:b + 1],
                                         scale=my[:, k, b:b + 1])
                    v.tensor_tensor(Bm[:, k, :], uB1[:], uB2[:],
                                    mybir.AluOpType.min)

                out_sb = outp.tile([H, 2, 3, IMG], F32, tag="out_sb", name="out_sb")
                for c in range(3):
                    # stage 1: psumT[yh] = img[:,:,c]^T @ A^T  (accum over xk)
                    Tbf = tbfp.tile([H, 2, IMG], BF16, tag="Tbf", name="Tbf")
                    for yh in range(2):
                        pT = ps_t.tile([H, IMG], F32, tag="pT", name="pT")
                        for xk in range(2):
                            nc.tensor.matmul(
                                pT[:],
                                img_bf[:, c, xk, H * yh:H * (yh + 1)],
                                AT[:, xk, :],
                                start=(xk == 0),
                                stop=(xk == 1),
                            )
                        drain_copy(Tbf[:, yh, :], pT[:])
                    # stage 2: out[jh] = T^T @ B (accum over yk)
                    for jh in range(2):
                        pO = ps_o.tile([H, IMG], F32, tag="pO", name="pO")
                        for yk in range(2):
                            nc.tensor.matmul(
                                pO[:],
                                Tbf[:, yk, H * jh:H * (jh + 1)],
                                Bm[:, yk, :],
                                start=(yk == 0),
                                stop=(yk == 1),
                            )
                        drain_copy(out_sb[:, jh, c, :], pO[:])

                for jh in range(2):
                    nc.sync.dma_start(
                        out_d.ap()[b].rearrange("c (jh p) jy -> p jh c jy",
                                                jh=2)[:, jh],
                        out_sb[:, jh],
                    )

    nc.compile()
    return nc


def _consts():
    jh = np.tile(((np.arange(IMG, dtype=np.float32) + np.float32(0.5))
                  / np.float32(112.0)).astype(np.float32), (H, 1))
    xc = np.empty((H, 2), np.float32)
    xc[:, 0] = np.arange(H, dtype=np.float32)
    xc[:, 1] = np.arange(H, dtype=np.float32) + H
    return jh, xc


def kernel(apn_out, inputs, _trace=False, _trace_kwargs=None):
    apn_out = np.ascontiguousarray(apn_out, dtype=np.float32)
    inputs = np.ascontiguousarray(inputs, dtype=np.float32)
    assert apn_out.shape == (B_FULL, 3) and inputs.shape == (B_FULL, 3, IMG, IMG)

    if "nc" not in _cache:
        _cache["nc"] = _build_graph()
    nc = _cache["nc"]

    jh, xc = _consts()
    in_maps = []
    for c in range(NCORES):
        sl = slice(c * BL, (c + 1) * BL)
        in_maps.append({
            "apn": apn_out[sl],
            "img": inputs[sl],
            "jh_mat": jh,
            "xcol": xc,
        })

    kwargs = {}
    if _trace:
        kwargs["trace"] = True
        if _trace_kwargs:
            kwargs.update(_trace_kwargs)
    res = run_bass_kernel_spmd(nc, in_maps, core_ids=list(range(NCORES)), **kwargs)
    _cache["last_result"] = res
    out = np.concatenate([res.results[c]["out"] for c in range(NCORES)], axis=0)
    return out


# revision 8
# speedup vs baseline: 4.0416x; 4.0416x over previous
"""AttentionCropLayer on 8 TRN2 NeuronCores, pure data-parallel.

Math: the whole layer is separable per image. With
  mx[x] = sigmoid(10(x-tx+tl)) - sigmoid(10(x-tx-tl))   (mask rows)
  my[y] likewise for ty, and the bilinear crop+resize being a sparse
  linear map per axis (two taps per output row), the output is
  out[c] = A @ img[c] @ B  where
  A[j,x]  = relu(1-|sA[j]-x|) * mx[x]   (tent = bilinear weights)
  B[y,jy] = relu(1-|sB[jy]-y|) * my[y]
  sA[j] = (tx-tl) + clamp((j+0.5)*2tl/224 - 0.5, 0, 2tl-1).
No data-dependent addressing: per-image matrices are built on-device with
elementwise ops from apn_out, then two bf16 matmul stages per channel.
"""

import sys

sys.path.insert(0, "/opt/trn_rl_repo")

import numpy as np

import concourse.bass as bass
import concourse.tile as tile
from concourse import bacc, mybir
from concourse.bass_utils import run_bass_kernel_spmd

F32 = mybir.dt.float32
BF16 = mybir.dt.bfloat16

NCORES = 8
B_FULL = 256
BL = B_FULL // NCORES  # 32 images per core
IMG = 224
H = 112  # chunk size (224 = 2*112)
MAGIC = 12582912.0  # 1.5 * 2^23: float32 round-to-int magic

_cache = {}


def _build_graph():
    nc = bacc.Bacc("TRN2", target_bir_lowering=False, debug=False)

    apn_d = nc.dram_tensor("apn", [BL, 3], F32, kind="ExternalInput")
    img_d = nc.dram_tensor("img", [BL, 3, IMG, IMG], F32, kind="ExternalInput")
    jh_d = nc.dram_tensor("jh_mat", [H, IMG], F32, kind="ExternalInput")
    xcol_d = nc.dram_tensor("xcol", [H, 2], F32, kind="ExternalInput")
    out_d = nc.dram_tensor("out", [BL, 3, IMG, IMG], F32, kind="ExternalOutput")

    with tile.TileContext(nc) as tc:
        with (
            tc.tile_pool(name="const", bufs=1) as constp,
            tc.tile_pool(name="param", bufs=1) as parp,
            tc.tile_pool(name="imgp", bufs=2) as imgp,
            tc.tile_pool(name="sbld", bufs=2) as sbld,
            tc.tile_pool(name="tent", bufs=3) as tentp,
            tc.tile_pool(name="mats", bufs=2) as matsp,
            tc.tile_pool(name="tbf", bufs=3) as tbfp,
            tc.tile_pool(name="outp", bufs=2) as outp,
            tc.tile_pool(name="ps_t", bufs=3, space=bass.MemorySpace.PSUM) as ps_t,
            tc.tile_pool(name="ps_o", bufs=3, space=bass.MemorySpace.PSUM) as ps_o,
        ):
            # ---- constants ----
            jh_mat = constp.tile([H, IMG], F32, tag="jh", name="jh")  # (j+0.5)/112, all parts
            nc.sync.dma_start(jh_mat[:], jh_d.ap())
            xcol = constp.tile([H, 2], F32, tag="xc", name="xc")  # xcol[p,k] = p + 112k
            nc.sync.dma_start(xcol[:], xcol_d.ap())
            nxcol = constp.tile([H, 2], F32, tag="nxc", name="nxc")

            # ---- per-image parameters, row layout [1, BL] ----
            v0 = nc.vector
            v0.tensor_scalar(nxcol[:], xcol[:], -1.0, None, mybir.AluOpType.mult,
                             mybir.AluOpType.bypass)
            apn_rows = []
            for pi in range(3):
                ar = parp.tile([1, BL], F32, tag=f"apn_r{pi}", name=f"apn_r{pi}")
                nc.sync.dma_start(ar[:], apn_d.ap().transpose([1, 0])[pi:pi + 1, :])
                apn_rows.append(ar)

            def row(tag):
                return parp.tile([1, BL], F32, tag=tag, name=tag)

            v = nc.vector

            def trunc_rows(dst, a_row, m0, m1, floor_only=False):
                # dst = trunc(op1(op0(a)))-style helper: dst = trunc(a*m0 + m1)
                u = row(dst + "_u")
                v.tensor_scalar(u[:], a_row, m0, m1, mybir.AluOpType.mult,
                                mybir.AluOpType.add)
                f = row(dst + "_f")
                v.tensor_scalar(f[:], u[:], 0.5, MAGIC, mybir.AluOpType.subtract,
                                mybir.AluOpType.add)
                v.tensor_scalar(f[:], f[:], MAGIC, None, mybir.AluOpType.subtract,
                                mybir.AluOpType.bypass)
                if not floor_only:
                    m = row(dst + "_m")
                    v.tensor_scalar(m[:], u[:], -1048576.0, 0.0,
                                    mybir.AluOpType.mult, mybir.AluOpType.max)
                    v.tensor_scalar(m[:], m[:], 1.0, None, mybir.AluOpType.min,
                                    mybir.AluOpType.bypass)
                    v.tensor_add(f[:], f[:], m[:])
                return f

            # tx = 112 + trunc(a0*56+0.5); ty likewise; tl = 38 + floor((a2+1)*9)
            tx = trunc_rows("tx", apn_rows[0][:], 56.0, 0.5)
            v.tensor_scalar(tx[:], tx[:], 112.0, None, mybir.AluOpType.add,
                            mybir.AluOpType.bypass)
            ty = trunc_rows("ty", apn_rows[1][:], 56.0, 0.5)
            v.tensor_scalar(ty[:], ty[:], 112.0, None, mybir.AluOpType.add,
                            mybir.AluOpType.bypass)
            # (a2+1)*9: op0=add 1, op1=mult 9 (order matters for f32 parity)
            tl_u = row("tl_u")
            v.tensor_scalar(tl_u[:], apn_rows[2][:], 1.0, 9.0, mybir.AluOpType.add,
                            mybir.AluOpType.mult)
            tl = row("tl")
            v.tensor_scalar(tl[:], tl_u[:], 0.5, MAGIC, mybir.AluOpType.subtract,
                            mybir.AluOpType.add)
            v.tensor_scalar(tl[:], tl[:], MAGIC, None, mybir.AluOpType.subtract,
                            mybir.AluOpType.bypass)
            v.tensor_scalar(tl[:], tl[:], 38.0, None, mybir.AluOpType.add,
                            mybir.AluOpType.bypass)

            stx = row("stx")
            v.tensor_sub(stx[:], tx[:], tl[:])
            enx = row("enx")
            v.tensor_add(enx[:], tx[:], tl[:])
            sty = row("sty")
            v.tensor_sub(sty[:], ty[:], tl[:])
            eny = row("eny")
            v.tensor_add(eny[:], ty[:], tl[:])
            t2m1 = row("t2m1")
            v.tensor_scalar(t2m1[:], tl[:], 2.0, 1.0, mybir.AluOpType.mult,
                            mybir.AluOpType.subtract)

            # ---- broadcast params to all 112 partitions ----
            def bcast(r, tag):
                t = parp.tile([H, BL], F32, tag=tag, name=tag)
                nc.gpsimd.partition_broadcast(t[:], r[:])
                return t

            TLb = bcast(tl, "TLb")
            T2b = bcast(t2m1, "T2b")
            STXb = bcast(stx, "STXb")
            ENXb = bcast(enx, "ENXb")
            STYb = bcast(sty, "STYb")
            ENYb = bcast(eny, "ENYb")

            # ---- mask columns mx/my [H, 2, BL]; plus negated copies ----
            def mask_cols(STb, ENb, tag):
                m = parp.tile([H, 2, BL], F32, tag=tag, name=tag)
                for k in range(2):
                    d1 = parp.tile([H, BL], F32, tag="mc_d1", name="mc_d1")
                    v.tensor_scalar(d1[:], STb[:], xcol[:, k:k + 1], -3.0,
                                    mybir.AluOpType.subtract, mybir.AluOpType.max)
                    v.tensor_scalar(d1[:], d1[:], 3.0, None, mybir.AluOpType.min,
                                    mybir.AluOpType.bypass)
                    g1 = parp.tile([H, BL], F32, tag="mc_g1", name="mc_g1")
                    nc.scalar.activation(g1[:], d1[:],
                                         mybir.ActivationFunctionType.Sigmoid,
                                         scale=-10.0)
                    d2 = parp.tile([H, BL], F32, tag="mc_d2", name="mc_d2")
                    v.tensor_scalar(d2[:], ENb[:], xcol[:, k:k + 1], -3.0,
                                    mybir.AluOpType.subtract, mybir.AluOpType.max)
                    v.tensor_scalar(d2[:], d2[:], 3.0, None, mybir.AluOpType.min,
                                    mybir.AluOpType.bypass)
                    g2 = parp.tile([H, BL], F32, tag="mc_g2", name="mc_g2")
                    nc.scalar.activation(g2[:], d2[:],
                                         mybir.ActivationFunctionType.Sigmoid,
                                         scale=-10.0)
                    v.tensor_sub(m[:, k, :], g1[:], g2[:])
                neg = parp.tile([H, 2, BL], F32, tag=tag + "_n", name=tag + "_n")
                v.tensor_scalar(neg[:], m[:], -1.0, None, mybir.AluOpType.mult,
                                mybir.AluOpType.bypass)
                return m, neg

            mx, nmx = mask_cols(STXb, ENXb, "mx")
            my, nmy = mask_cols(STYb, ENYb, "my")

            # ---- per-image pipeline ----
            drain_ctr = [0]

            def drain_copy(out_ap, in_ap):
                # PSUM drains: DVE-heavy split (ACT is tent-loaded)
                drain_ctr[0] += 1
                if drain_ctr[0] % 3 != 0:
                    nc.vector.tensor_copy(out_ap, in_ap)
                else:
                    nc.scalar.activation(out_ap, in_ap,
                                         mybir.ActivationFunctionType.Copy)

            for b in range(BL):
                # image load + bf16 convert (gpsimd)
                img_f = imgp.tile([H, 3, 2, IMG], F32, tag="img_f", name="img_f")
                nc.sync.dma_start(
                    img_f[:],
                    img_d.ap()[b].rearrange("c (k p) y -> p c k y", k=2),
                )
                img_bf = imgp.tile([H, 3, 2, IMG], BF16, tag="img_bf", name="img_bf")
                nc.vector.tensor_copy(img_bf[:], img_f[:])

                # s rows (shared u), built on gpsimd: [H, IMG] broadcast form
                uu = sbld.tile([H, IMG], F32, tag="uu", name="uu")
                v.tensor_scalar(uu[:], jh_mat[:], TLb[:, b:b + 1], 0.5,
                                mybir.AluOpType.mult,
                                mybir.AluOpType.subtract)
                v.tensor_scalar(uu[:], uu[:], 0.0, None,
                                mybir.AluOpType.max,
                                mybir.AluOpType.bypass)
                sA = sbld.tile([H, IMG], F32, tag="sA", name="sA")
                v.tensor_scalar(sA[:], uu[:], T2b[:, b:b + 1],
                                STXb[:, b:b + 1], mybir.AluOpType.min,
                                mybir.AluOpType.add)
                sB = sbld.tile([H, IMG], F32, tag="sB", name="sB")
                v.tensor_scalar(sB[:], uu[:], T2b[:, b:b + 1],
                                STYb[:, b:b + 1], mybir.AluOpType.min,
                                mybir.AluOpType.add)

                # tents + mask fold -> A^T [H,2,IMG] bf16, B [H,2,IMG] bf16
                AT = matsp.tile([H, 2, IMG], BF16, tag="AT", name="AT")
                Bm = matsp.tile([H, 2, IMG], BF16, tag="Bm", name="Bm")
                for k in range(2):
                    # |s - x| on ACT via Abs(in + (-x)); then
                    # tent*m = Relu(|d|*(-m) + m), bf16 out. 2 ACT ops, 0 DVE.
                    aA = tentp.tile([H, IMG], F32, tag="aA", name="aA")
                    nc.scalar.activation(aA[:], sA[:],
                                         mybir.ActivationFunctionType.Abs,
                                         bias=nxcol[:, k:k + 1])
                    nc.scalar.activation(AT[:, k, :], aA[:],
                                         mybir.ActivationFunctionType.Relu,
                                         bias=mx[:, k, b:b + 1],
                                         scale=nmx[:, k, b:b + 1])
                    aB = tentp.tile([H, IMG], F32, tag="aB", name="aB")
                    nc.scalar.activation(aB[:], sB[:],
                                         mybir.ActivationFunctionType.Abs,
                                         bias=nxcol[:, k:k + 1])
                    nc.scalar.activation(Bm[:, k, :], aB[:],
                                         mybir.ActivationFunctionType.Relu,
                                         bias=my[:, k, b:b + 1],
                                         scale=nmy[:, k, b:b + 1])

                out_sb = outp.tile([H, 3, 2, IMG], F32, tag="out_sb", name="out_sb")
                for c in range(3):
                    # stage 1: psumT[:, yh, :] = img[:,:,c]^T @ A^T (accum xk);
                    # both yh halves share one PSUM bank -> single drain
                    Tbf = tbfp.tile([H, 2, IMG], BF16, tag="Tbf", name="Tbf")
                    pT = ps_t.tile([H, 2, IMG], F32, tag="pT", name="pT")
                    for yh in range(2):
                        for xk in range(2):
                            nc.tensor.matmul(
                                pT[:, yh, :],
                                img_bf[:, c, xk, H * yh:H * (yh + 1)],
                                AT[:, xk, :],
                                start=(xk == 0),
                                stop=(xk == 1),
                            )
                    drain_copy(Tbf[:], pT[:])
                    # stage 2: out[:, jh, :] = T^T @ B (accum yk), one bank
                    pO = ps_o.tile([H, 2, IMG], F32, tag="pO", name="pO")
                    for jh in range(2):
                        for yk in range(2):
                            nc.tensor.matmul(
                                pO[:, jh, :],
                                Tbf[:, yk, H * jh:H * (jh + 1)],
                                Bm[:, yk, :],
                                start=(yk == 0),
                                stop=(yk == 1),
                            )
                    drain_copy(out_sb[:, c], pO[:])
                    nc.sync.dma_start(
                        out_d.ap()[b].rearrange("c (jh p) jy -> p c jh jy",
                                                jh=2)[:, c],
                        out_sb[:, c],
                    )

    nc.compile()
    return nc


def _consts():
    jh = np.tile(((np.arange(IMG, dtype=np.float32) + np.float32(0.5))
                  / np.float32(112.0)).astype(np.float32), (H, 1))
    xc = np.empty((H, 2), np.float32)
    xc[:, 0] = np.arange(H, dtype=np.float32)
    xc[:, 1] = np.arange(H, dtype=np.float32) + H
    return jh, xc


def kernel(apn_out, inputs, _trace=False, _trace_kwargs=None):
    apn_out = np.ascontiguousarray(apn_out, dtype=np.float32)
    inputs = np.ascontiguousarray(inputs, dtype=np.float32)
    assert apn_out.shape == (B_FULL, 3) and inputs.shape == (B_FULL, 3, IMG, IMG)

    if "nc" not in _cache:
        _cache["nc"] = _build_graph()
    nc = _cache["nc"]

    jh, xc = _consts()
    in_maps = []
    for c in range(NCORES):
        sl = slice(c * BL, (c + 1) * BL)
        in_maps.append({
            "apn": apn_out[sl],
            "img": inputs[sl],
            "jh_mat": jh,
            "xcol": xc,
        })

    kwargs = {}
    if _trace:
        kwargs["trace"] = True
        if _trace_kwargs:
            kwargs.update(_trace_kwargs)
    res = run_bass_kernel_spmd(nc, in_maps, core_ids=list(range(NCORES)), **kwargs)
    _cache["last_result"] = res
    out = np.concatenate([res.results[c]["out"] for c in range(NCORES)], axis=0)
    return out
